# revision 12
# baseline (speedup 1.0000x reference)
"""Canny NMS kernel V13e for trn2, 8-core data parallel.

Structure vs V11 (280us baseline):
- u/d row shifts of msq2 via SBUF->SBUF partition-shifted DMA (DMA is
  exempt from the start-partition {0,32,64,96} rule) instead of PE f16
  identity matmuls + Act copies: -4.6us Act/group, -PSUM serialization.
- scalar_tensor_tensor ops (r1, blur, sht chains) decomposed into Pool
  tensor_tensor mult-by-broadcast-const + add: moves ~6.6us/group from
  DVE (the bottleneck) to Pool (which only supports f32 tt arith).
- msq = gx^2+gy^2 from two f32 Act squares + one Pool add; msq2 =
  Square(msq)->f16 (Act). Masks from single-rounded f16 scaled squares
  (Act): mA16=(T1gx)^2, mB16=(T2gx)^2, c16=gy^2; m0/m90 DVE f16 tt.
- Engine split/group: Pool 11 f32 tt (18.8us) | Act 8 (18us) | DVE:
  m0,m90,pxy,s45,4 maxes,3 copy_predicated,z (17.5us) | PE 20 fp32 mm
  (17.1us).  (fp32r matmuls tested: 4x faster but E8M11 rounding of
  sht explodes flips via gy cancellation -- rejected.)
- Last 28 output rows (484..512) as a "mini" group: 4 imgs x 32-row
  blocks on the partition dim, free dim 512+pads (~1/4 group cost);
  block-diagonal band matrices; block-end zero guards via DMA from a
  zeros dram tensor.
- Pipeline: back(g-1); front(g); load(g+1); single-buffered NMS tiles.
"""

import numpy as np

H = W = 512
B = 32
N_CORES = 8
IMGS_PER_CORE = B // N_CORES  # 4

# full groups: (s, load_r0, load_r1, z_p0, z_p1)
GROUPS = [
    (0, 0, 128, 0, 124),      # z rows [0, 124)
    (120, 120, 248, 4, 124),  # [124, 244)
    (240, 240, 368, 4, 124),  # [244, 364)
    (360, 360, 488, 4, 124),  # [364, 484)
]
MINI_R0 = 480                 # mini group: rows 480..512, z [484, 512)
NG = 5

_GRAY_W = (0.299, 0.587, 0.114)
T1 = float(np.float32(np.tan(np.deg2rad(22.5))))
T2 = float(np.float32(np.tan(np.deg2rad(67.5))))

NI = IMGS_PER_CORE  # 4
F = NI * W          # 2048
HF = F // 2         # 1024
W2, W4 = W + 2, W + 4
FP = NI * W2        # 2056
FP2 = NI * W4       # 2064

_NC_CACHE = {}


def _g1n():
    ax = np.arange(-2, 3, dtype=np.float64)
    g = np.exp(-(ax * ax) / 2.0)
    return g / g.sum()


def _band(w, off, n=128):
    Bm = np.zeros((n, n), np.float32)
    idx = np.arange(n)
    for d, wv in enumerate(w):
        kk = idx + d - off
        valid = (kk >= 0) & (kk < n)
        Bm[kk[valid], idx[valid]] = np.float32(wv)
    return Bm


def _band_blocks(w, off, nb=4, bs=32):
    """Block-diagonal band matrix: nb independent bs-row blocks."""
    Bm = np.zeros((nb * bs, nb * bs), np.float32)
    blk = _band(w, off, bs)
    for b in range(nb):
        Bm[b * bs:(b + 1) * bs, b * bs:(b + 1) * bs] = blk
    return Bm


def _weights32():
    g1 = _g1n()
    return np.stack([
        _band(_GRAY_W[0] * g1, 2),          # 0: ch0 gray+Gv
        _band(_GRAY_W[1] * g1, 2),          # 1: ch1
        _band(_GRAY_W[2] * g1, 2),          # 2: ch2
        _band([1.0, 2.0, 1.0], 1),          # 3: Sv (gx vertical)
        _band([-1.0, 0.0, 1.0], 1),         # 4: Dv (gy vertical)
        _band_blocks(_GRAY_W[0] * g1, 2),   # 5: mini ch0
        _band_blocks(_GRAY_W[1] * g1, 2),   # 6: mini ch1
        _band_blocks(_GRAY_W[2] * g1, 2),   # 7: mini ch2
        _band_blocks([1.0, 2.0, 1.0], 1),   # 8: mini Sv
        _band_blocks([-1.0, 0.0, 1.0], 1),  # 9: mini Dv
    ]).astype(np.float32)


def _build(n_reps):
    import concourse.bacc as bacc
    import concourse.tile as tile
    from concourse import mybir

    f32 = mybir.dt.float32
    f16 = mybir.dt.float16
    u16 = mybir.dt.uint16
    Alu = mybir.AluOpType
    Act = mybir.ActivationFunctionType

    g1 = _g1n()
    a_ov_b = float(np.float32(g1[0] / g1[1]))
    b_ov_c = float(np.float32(g1[1] / g1[2]))

    nc = bacc.Bacc("TRN2", target_bir_lowering=False, debug=False,
                   num_devices=N_CORES)
    x_d = nc.dram_tensor("x", [NI, 3, H, W], f32,
                         kind="ExternalInput").ap()
    w32_d = nc.dram_tensor("w32", [10, 128, 128], f32,
                           kind="ExternalInput").ap()
    z_d = nc.dram_tensor("zeros", [8, 3 * F], f32, kind="ExternalInput").ap()
    y_d = nc.dram_tensor("y", [NI, H, W], f16, kind="ExternalOutput").ap()

    with tile.TileContext(nc) as tc:
        import contextlib
        with contextlib.ExitStack() as ctx:
            wpool = ctx.enter_context(tc.tile_pool(name="w", bufs=1))
            sb = ctx.enter_context(tc.tile_pool(name="sb", bufs=1))
            ps = ctx.enter_context(tc.tile_pool(name="ps", bufs=1,
                                                space="PSUM"))

            wt32 = wpool.tile([128, 10 * 128], f32)
            nc.sync.dma_start(
                wt32[:].rearrange("k (n m) -> k n m", n=10),
                w32_d.rearrange("n k m -> k n m"))

            def wsl(n):
                return wt32[:, n * 128:(n + 1) * 128]

            # broadcast scalar constants for Pool tt mults
            cbt = wpool.tile([128, 2], f32)

            def bc(k, shape):
                return cbt[:, k:k + 1].to_broadcast(shape)

            # --- SBUF tiles ---
            xts = [sb.tile([128, 3 * F], f32, name=f"xt{i}")
                   for i in range(2)]
            Asb = sb.tile([128, FP2], f32, name="Asb")
            blurT = sb.tile([128, FP], f32, name="blurT")
            s1 = sb.tile([128, F], f32, name="s1")
            s2 = sb.tile([128, F], f32, name="s2")
            s3 = sb.tile([128, F], f32, name="s3")
            s4 = sb.tile([128, F], f32, name="s4")
            gy16 = sb.tile([128, F], f16, name="gy16")
            a32 = sb.tile([128, F], f32, name="a32")
            c32 = sb.tile([128, F], f32, name="c32")
            mA16 = sb.tile([128, F], f16, name="mA16")
            mB16 = sb.tile([128, F], f16, name="mB16")
            c16 = sb.tile([128, F], f16, name="c16")
            pxyt = sb.tile([128, F], f16, name="pxyt")
            msq2t = sb.tile([128, FP], f16, name="msq2t")
            usbt = sb.tile([128, FP], f16, name="usbt")
            dsbt = sb.tile([128, FP], f16, name="dsbt")
            m0t = sb.tile([128, F], u16, name="m0t")
            m90t = sb.tile([128, F], u16, name="m90t")
            s45t = sb.tile([128, F], u16, name="s45t")
            mselt = sb.tile([128, F], f16, name="mselt")
            m45v = sb.tile([128, F], f16, name="m45v")
            m90v = sb.tile([128, F], f16, name="m90v")
            m0v = sb.tile([128, F], f16, name="m0v")

            # --- PSUM: psA x2 halves + psX + psY = 8 banks ---
            psA = [ps.tile([128, HF], f32, name=f"psA{i}")
                   for i in range(2)]
            psX = ps.tile([128, HF], f32, name="psX")
            psY = ps.tile([128, HF], f32, name="psY")

            def pv(t, wpad, lo, hi):
                return t[:].rearrange("p (i w) -> p i w", i=NI)[:, :, lo:hi]

            def fv(ap):
                return ap.rearrange("p (i w) -> p i w", i=NI)

            # --- hoisted one-time setup (outside rep loop) ---
            nc.gpsimd.memset(cbt[:, 0:1], a_ov_b)
            nc.gpsimd.memset(cbt[:, 1:2], b_ov_c)
            nc.gpsimd.memset(pv(Asb, W4, 0, 2), 0.0)
            nc.gpsimd.memset(pv(Asb, W4, W + 2, W + 4), 0.0)
            nc.gpsimd.memset(pv(blurT, W2, 0, 1), 0.0)
            nc.gpsimd.memset(pv(blurT, W2, W + 1, W + 2), 0.0)
            for t in (msq2t, usbt, dsbt):
                nc.gpsimd.memset(pv(t, W2, 0, 1), 0.0)
                nc.gpsimd.memset(pv(t, W2, W + 1, W + 2), 0.0)
            # usbt row 0 = image row -1 for group 0 (shift DMAs write
            # usbt[1:128] only, so it stays zero)
            nc.gpsimd.memset(usbt[0:1, :], 0.0)
            # dsbt row 127 is never written by the main shift
            nc.gpsimd.memset(dsbt[96:128, :], 0.0)

            zrow16 = z_d[0:4, :].bitcast(f16)

            def load(gi):
                ph = gi % 2
                xt = xts[ph]
                if gi < 4:
                    s, r0, r1, _, _ = GROUPS[gi]
                    for c in range(3):
                        nc.sync.dma_start(
                            xt[0:r1 - r0, c * F:(c + 1) * F].rearrange(
                                "p (i w) -> p i w", i=NI),
                            x_d.rearrange("i c h w -> c h i w")[c, r0:r1])
                else:
                    # mini: rows 480..512, partitions = (img, row-in-32)
                    for c in range(3):
                        for i in range(4):
                            nc.sync.dma_start(
                                xt[32 * i:32 * (i + 1),
                                   c * W:(c + 1) * W],
                                x_d[i, c, MINI_R0:H, :])

            def consume_psum(hs, pX, pY):
                """Act/DVE consumption of gx/gy PSUM tiles for one half."""
                nc.scalar.activation(a32[:, hs], pX, Act.Square)
                nc.scalar.activation(c32[:, hs], pY, Act.Square)
                nc.scalar.activation(mA16[:, hs], pX, Act.Square, scale=T1)
                nc.scalar.activation(mB16[:, hs], pX, Act.Square, scale=T2)
                nc.scalar.activation(c16[:, hs], pY, Act.Square)
                nc.scalar.copy(gy16[:, hs], pY)
                nc.vector.tensor_tensor(pxyt[:, hs], pX, gy16[:, hs],
                                        op=Alu.mult)
                nc.vector.tensor_tensor(m0t[:, hs], mA16[:, hs],
                                        c16[:, hs], op=Alu.is_gt)
                nc.vector.tensor_tensor(m90t[:, hs], mB16[:, hs],
                                        c16[:, hs], op=Alu.is_le)
                # msq (Pool)
                nc.gpsimd.tensor_tensor(s4[:, hs], a32[:, hs],
                                        c32[:, hs], op=Alu.add)

            def blur_chain(asb_taps, blur_out, blur_taps, n, w):
                """Pool-only blurH + sobel-horizontal chains.

                asb_taps: [m2, m1, c0, p1, p2] views of padded A
                blur_taps: [m1, c0, p1] views of padded blur
                w(tile): view of a flat scratch tile shaped like the taps
                (fv for full groups, [:, 0:W] for the mini group).
                dxh lands in s1, sht in s3.
                """
                m2, m1, c0, p1, p2 = asb_taps
                nc.gpsimd.tensor_tensor(w(s1), m2, p2, op=Alu.add)
                nc.gpsimd.tensor_tensor(w(s2), m1, p1, op=Alu.add)
                nc.gpsimd.tensor_tensor(s3[:, 0:n], s1[:, 0:n],
                                        bc(0, [128, n]), op=Alu.mult)
                nc.gpsimd.tensor_tensor(s1[:, 0:n], s3[:, 0:n],
                                        s2[:, 0:n], op=Alu.add)
                nc.gpsimd.tensor_tensor(s3[:, 0:n], s1[:, 0:n],
                                        bc(1, [128, n]), op=Alu.mult)
                bm1, bc0, bp1 = blur_taps
                nc.gpsimd.tensor_tensor(blur_out, w(s3), c0, op=Alu.add)
                # sobel horizontal
                nc.gpsimd.tensor_tensor(w(s1), bp1, bm1,
                                        op=Alu.subtract)          # dxh
                nc.gpsimd.tensor_tensor(w(s2), bm1, bp1, op=Alu.add)
                nc.gpsimd.tensor_tensor(w(s4), w(s2), bc0, op=Alu.add)
                nc.gpsimd.tensor_tensor(w(s3), w(s4), bc0, op=Alu.add)

            def front_full(gi):
                ph = gi % 2
                xt = xts[ph]
                # ---- A = (gray o Gv) band mms, halves
                for h in range(2):
                    for i2 in range(2):
                        i = 2 * h + i2
                        for c in range(3):
                            nc.tensor.matmul(
                                psA[h][:, i2 * W:(i2 + 1) * W], wsl(c),
                                xt[:, c * F + i * W:c * F + (i + 1) * W],
                                start=(c == 0), stop=(c == 2))
                    nc.scalar.copy(
                        pv(Asb, W4, 2, W + 2)[:, 2 * h:2 * h + 2, :],
                        psA[h][:].rearrange("p (i w) -> p i w", i=2))

                blur_chain(
                    [pv(Asb, W4, 0, W), pv(Asb, W4, 1, W + 1),
                     pv(Asb, W4, 2, W + 2), pv(Asb, W4, 3, W + 3),
                     pv(Asb, W4, 4, W + 4)],
                    pv(blurT, W2, 1, W + 1),
                    [pv(blurT, W2, 0, W), pv(blurT, W2, 1, W + 1),
                     pv(blurT, W2, 2, W + 2)],
                    F, lambda t: fv(t[:]))

                # ---- gx/gy mms + PSUM consumption per half
                for h in range(2):
                    hs = slice(h * HF, (h + 1) * HF)
                    for i2 in range(2):
                        i = 2 * h + i2
                        nc.tensor.matmul(
                            psX[:, i2 * W:(i2 + 1) * W], wsl(3),
                            s1[:, i * W:(i + 1) * W],
                            start=True, stop=True)
                        nc.tensor.matmul(
                            psY[:, i2 * W:(i2 + 1) * W], wsl(4),
                            s3[:, i * W:(i + 1) * W],
                            start=True, stop=True)
                    consume_psum(hs, psX[:], psY[:])
                # msq2 (Act) + u/d shift DMAs + s45
                nc.scalar.activation(pv(msq2t, W2, 1, W + 1),
                                     fv(s4[:]), Act.Square)
                nc.sync.dma_start(usbt[1:128, :], msq2t[0:127, :])
                nc.sync.dma_start(dsbt[0:127, :], msq2t[1:128, :])
                nc.vector.tensor_scalar(s45t[:], pxyt[:], 0.0, None,
                                        op0=Alu.is_ge)

            def front_mini(gi):
                ph = gi % 2
                xt = xts[ph]
                Am = Asb[:, 0:W4]
                blm = blurT[:, 0:W2]
                psAm = psA[0][:, 0:W]
                psXm = psX[:, 0:W]
                psYm = psY[:, 0:W]
                for c in range(3):
                    nc.tensor.matmul(
                        psAm, wsl(5 + c), xt[:, c * W:(c + 1) * W],
                        start=(c == 0), stop=(c == 2))
                nc.scalar.copy(Am[:, 2:W + 2], psAm)
                blur_chain(
                    [Am[:, 0:W], Am[:, 1:W + 1], Am[:, 2:W + 2],
                     Am[:, 3:W + 3], Am[:, 4:W + 4]],
                    blm[:, 1:W + 1],
                    [blm[:, 0:W], blm[:, 1:W + 1], blm[:, 2:W + 2]],
                    W, lambda t: t[:, 0:W])
                nc.tensor.matmul(psXm, wsl(8), s1[:, 0:W],
                                 start=True, stop=True)
                nc.tensor.matmul(psYm, wsl(9), s3[:, 0:W],
                                 start=True, stop=True)
                consume_psum(slice(0, W), psXm, psYm)
                nc.scalar.activation(msq2t[:, 1:W + 1], s4[:, 0:W],
                                     Act.Square)
                nc.sync.dma_start(usbt[1:128, 0:W2], msq2t[0:127, 0:W2])
                # d-shift per 32-row block; block-end rows re-zeroed
                for b in range(4):
                    nc.sync.dma_start(dsbt[32 * b:32 * b + 31, 0:W2],
                                      msq2t[32 * b + 1:32 * b + 32, 0:W2])
                    nc.sync.dma_start(dsbt[32 * b + 31:32 * b + 32, 0:W2],
                                      zrow16[b:b + 1, 0:W2])
                nc.vector.tensor_scalar(s45t[:, 0:W], pxyt[:, 0:W],
                                        0.0, None, op0=Alu.is_ge)

            def back(gi):
                if gi < 4:
                    s, r0, r1, zp0, zp1 = GROUPS[gi]
                    nc.vector.tensor_tensor(
                        fv(mselt[:]), pv(usbt, W2, 0, W),
                        pv(dsbt, W2, 2, W + 2), op=Alu.max)  # ul/dr (135)
                    nc.vector.tensor_tensor(
                        fv(m45v[:]), pv(dsbt, W2, 0, W),
                        pv(usbt, W2, 2, W + 2), op=Alu.max)  # dl/ur (45)
                    nc.vector.tensor_tensor(
                        fv(m90v[:]), pv(usbt, W2, 1, W + 1),
                        pv(dsbt, W2, 1, W + 1), op=Alu.max)
                    nc.vector.tensor_tensor(
                        fv(m0v[:]), pv(msq2t, W2, 0, W),
                        pv(msq2t, W2, 2, W + 2), op=Alu.max)
                    nc.vector.copy_predicated(mselt[:], s45t[:], m45v[:])
                    nc.vector.copy_predicated(mselt[:], m90t[:], m90v[:])
                    nc.vector.copy_predicated(mselt[:], m0t[:], m0v[:])
                    nc.vector.tensor_tensor(
                        fv(m45v[:]), pv(msq2t, W2, 1, W + 1),
                        fv(mselt[:]), op=Alu.is_ge)
                    zr0, zr1 = s + zp0, s + zp1
                    for i in range(NI):
                        nc.sync.dma_start(
                            y_d[i, zr0:zr1, :],
                            m45v[zp0:zp1, i * W:(i + 1) * W])
                else:
                    nc.vector.tensor_tensor(
                        mselt[:, 0:W], usbt[:, 0:W], dsbt[:, 2:W + 2],
                        op=Alu.max)
                    nc.vector.tensor_tensor(
                        m45v[:, 0:W], dsbt[:, 0:W], usbt[:, 2:W + 2],
                        op=Alu.max)
                    nc.vector.tensor_tensor(
                        m90v[:, 0:W], usbt[:, 1:W + 1], dsbt[:, 1:W + 1],
                        op=Alu.max)
                    nc.vector.tensor_tensor(
                        m0v[:, 0:W], msq2t[:, 0:W], msq2t[:, 2:W + 2],
                        op=Alu.max)
                    nc.vector.copy_predicated(mselt[:, 0:W], s45t[:, 0:W],
                                              m45v[:, 0:W])
                    nc.vector.copy_predicated(mselt[:, 0:W], m90t[:, 0:W],
                                              m90v[:, 0:W])
                    nc.vector.copy_predicated(mselt[:, 0:W], m0t[:, 0:W],
                                              m0v[:, 0:W])
                    nc.vector.tensor_tensor(
                        m45v[:, 0:W], msq2t[:, 1:W + 1], mselt[:, 0:W],
                        op=Alu.is_ge)
                    for i in range(NI):
                        nc.sync.dma_start(
                            y_d[i, MINI_R0 + 4:H, :],
                            m45v[32 * i + 4:32 * (i + 1), 0:W])

            def front(gi):
                if gi < 4:
                    front_full(gi)
                else:
                    front_mini(gi)

            import contextlib as _ctl
            rep_ctx = (tc.For_i(0, n_reps, 1) if n_reps > 1
                       else _ctl.nullcontext())
            with rep_ctx:
                # pipeline: B(g-1); F(g); load(g+1)
                load(0)
                for g in range(NG + 1):
                    if g >= 1:
                        back(g - 1)
                    if g < NG:
                        front(g)
                    if g + 1 < NG:
                        load(g + 1)
    nc.compile()
    return nc


def _get_nc(n_reps):
    if n_reps not in _NC_CACHE:
        _NC_CACHE[n_reps] = _build(n_reps)
    return _NC_CACHE[n_reps]


def run_on_cores(x, n_reps=1):
    from concourse.bass_utils import run_bass_kernel_spmd

    nc = _get_nc(n_reps)
    w32 = _weights32()
    zeros = np.zeros((8, 3 * F), np.float32)
    x = np.ascontiguousarray(np.asarray(x), dtype=np.float32)
    in_maps = [
        {"x": x[c * IMGS_PER_CORE:(c + 1) * IMGS_PER_CORE],
         "w32": w32, "zeros": zeros}
        for c in range(N_CORES)
    ]
    res = run_bass_kernel_spmd(nc, in_maps, list(range(N_CORES)))
    out = np.concatenate(
        [np.asarray(res.results[c]["y"], dtype=np.float32)[:, None]
         for c in range(N_CORES)], axis=0)
    return out


def kernel(x):
    return run_on_cores(x, n_reps=1)


# revision 13
# speedup vs baseline: 1.1552x; 1.1552x over previous
"""Canny NMS kernel V13e for trn2, 8-core data parallel.

Structure vs V11 (280us baseline):
- u/d row shifts of msq2 via SBUF->SBUF partition-shifted DMA (DMA is
  exempt from the start-partition {0,32,64,96} rule) instead of PE f16
  identity matmuls + Act copies: -4.6us Act/group, -PSUM serialization.
- scalar_tensor_tensor ops (r1, blur, sht chains) decomposed into Pool
  tensor_tensor mult-by-broadcast-const + add: moves ~6.6us/group from
  DVE (the bottleneck) to Pool (which only supports f32 tt arith).
- msq = gx^2+gy^2 from two f32 Act squares + one Pool add; msq2 =
  Square(msq)->f16 (Act). Masks from single-rounded f16 scaled squares
  (Act): mA16=(T1gx)^2, mB16=(T2gx)^2, c16=gy^2; m0/m90 DVE f16 tt.
- Engine split/group: Pool 11 f32 tt (18.8us) | Act 8 (18us) | DVE:
  m0,m90,pxy,s45,4 maxes,3 copy_predicated,z (17.5us) | PE 20 fp32 mm
  (17.1us).  (fp32r matmuls tested: 4x faster but E8M11 rounding of
  sht explodes flips via gy cancellation -- rejected.)
- Last 28 output rows (484..512) as a "mini" group: 4 imgs x 32-row
  blocks on the partition dim, free dim 512+pads (~1/4 group cost);
  block-diagonal band matrices; block-end zero guards via DMA from a
  zeros dram tensor.
- Pipeline: back(g-1); front(g); load(g+1); single-buffered NMS tiles.
"""

import numpy as np

H = W = 512
B = 32
N_CORES = 8
IMGS_PER_CORE = B // N_CORES  # 4

# full groups: (s, load_r0, load_r1, z_p0, z_p1)
GROUPS = [
    (0, 0, 128, 0, 124),      # z rows [0, 124)
    (120, 120, 248, 4, 124),  # [124, 244)
    (240, 240, 368, 4, 124),  # [244, 364)
    (360, 360, 488, 4, 124),  # [364, 484)
]
MINI_R0 = 480                 # mini group: rows 480..512, z [484, 512)
NG = 5

_GRAY_W = (0.299, 0.587, 0.114)
T1 = float(np.float32(np.tan(np.deg2rad(22.5))))
T2 = float(np.float32(np.tan(np.deg2rad(67.5))))

NI = IMGS_PER_CORE  # 4
F = NI * W          # 2048
HF = F // 2         # 1024
W2, W4 = W + 2, W + 4
FP = NI * W2        # 2056
FP2 = NI * W4       # 2064

_NC_CACHE = {}


def _g1n():
    ax = np.arange(-2, 3, dtype=np.float64)
    g = np.exp(-(ax * ax) / 2.0)
    return g / g.sum()


def _band(w, off, n=128):
    Bm = np.zeros((n, n), np.float32)
    idx = np.arange(n)
    for d, wv in enumerate(w):
        kk = idx + d - off
        valid = (kk >= 0) & (kk < n)
        Bm[kk[valid], idx[valid]] = np.float32(wv)
    return Bm


def _band_blocks(w, off, nb=4, bs=32):
    """Block-diagonal band matrix: nb independent bs-row blocks."""
    Bm = np.zeros((nb * bs, nb * bs), np.float32)
    blk = _band(w, off, bs)
    for b in range(nb):
        Bm[b * bs:(b + 1) * bs, b * bs:(b + 1) * bs] = blk
    return Bm


def _weights32():
    g1 = _g1n()
    return np.stack([
        _band(_GRAY_W[0] * g1, 2),          # 0: ch0 gray+Gv
        _band(_GRAY_W[1] * g1, 2),          # 1: ch1
        _band(_GRAY_W[2] * g1, 2),          # 2: ch2
        _band([1.0, 2.0, 1.0], 1),          # 3: Sv (gx vertical)
        _band([-1.0, 0.0, 1.0], 1),         # 4: Dv (gy vertical)
        _band_blocks(_GRAY_W[0] * g1, 2),   # 5: mini ch0
        _band_blocks(_GRAY_W[1] * g1, 2),   # 6: mini ch1
        _band_blocks(_GRAY_W[2] * g1, 2),   # 7: mini ch2
        _band_blocks([1.0, 2.0, 1.0], 1),   # 8: mini Sv
        _band_blocks([-1.0, 0.0, 1.0], 1),  # 9: mini Dv
    ]).astype(np.float32)


def _weights16():
    return np.stack([
        _band([1.0], 1),               # 0: shift up   u[m] = in[m-1]
        _band([1.0], -1),              # 1: shift down d[m] = in[m+1]
        _band_blocks([1.0], 1),        # 2: mini shift up (block-diag)
        _band_blocks([1.0], -1),       # 3: mini shift down (block-diag)
    ]).astype(np.float16)


def _build(n_reps):
    import concourse.bacc as bacc
    import concourse.tile as tile
    from concourse import mybir

    f32 = mybir.dt.float32
    f16 = mybir.dt.float16
    u16 = mybir.dt.uint16
    Alu = mybir.AluOpType
    Act = mybir.ActivationFunctionType

    g1 = _g1n()
    a_ov_b = float(np.float32(g1[0] / g1[1]))
    b_ov_c = float(np.float32(g1[1] / g1[2]))

    nc = bacc.Bacc("TRN2", target_bir_lowering=False, debug=False,
                   num_devices=N_CORES)
    x_d = nc.dram_tensor("x", [NI, 3, H, W], f32,
                         kind="ExternalInput").ap()
    w32_d = nc.dram_tensor("w32", [10, 128, 128], f32,
                           kind="ExternalInput").ap()
    w16_d = nc.dram_tensor("w16", [4, 128, 128], f16,
                           kind="ExternalInput").ap()
    z_d = nc.dram_tensor("zeros", [8, 3 * F], f32, kind="ExternalInput").ap()
    y_d = nc.dram_tensor("y", [NI, H, W], f16, kind="ExternalOutput").ap()

    with tile.TileContext(nc) as tc:
        import contextlib
        with contextlib.ExitStack() as ctx:
            wpool = ctx.enter_context(tc.tile_pool(name="w", bufs=1))
            sb = ctx.enter_context(tc.tile_pool(name="sb", bufs=1))
            ps = ctx.enter_context(tc.tile_pool(name="ps", bufs=1,
                                                space="PSUM"))

            wt32 = wpool.tile([128, 10 * 128], f32)
            nc.sync.dma_start(
                wt32[:].rearrange("k (n m) -> k n m", n=10),
                w32_d.rearrange("n k m -> k n m"))

            def wsl(n):
                return wt32[:, n * 128:(n + 1) * 128]

            wt16 = wpool.tile([128, 4 * 128], f16)
            nc.sync.dma_start(
                wt16[:].rearrange("k (n m) -> k n m", n=4),
                w16_d.rearrange("n k m -> k n m"))

            def wsl16(n):
                return wt16[:, n * 128:(n + 1) * 128]

            # broadcast scalar constants for Pool tt mults
            cbt = wpool.tile([128, 2], f32)

            def bc(k, shape):
                return cbt[:, k:k + 1].to_broadcast(shape)

            # --- SBUF tiles ---
            xts = [sb.tile([128, 3 * F], f32, name=f"xt{i}")
                   for i in range(2)]
            Asb = sb.tile([128, FP2], f32, name="Asb")
            blurT = sb.tile([128, FP], f32, name="blurT")
            s1 = sb.tile([128, F], f32, name="s1")
            s2 = sb.tile([128, F], f32, name="s2")
            s3 = sb.tile([128, F], f32, name="s3")
            s4 = sb.tile([128, F], f32, name="s4")
            gy16 = sb.tile([128, F], f16, name="gy16")
            a32 = sb.tile([128, F], f32, name="a32")
            c32 = sb.tile([128, F], f32, name="c32")
            mA16 = sb.tile([128, F], f16, name="mA16")
            mB16 = sb.tile([128, F], f16, name="mB16")
            c16 = sb.tile([128, F], f16, name="c16")
            pxyt = sb.tile([128, F], f16, name="pxyt")
            msq2t = sb.tile([128, FP], f16, name="msq2t")
            usbt = sb.tile([128, FP], f16, name="usbt")
            dsbt = sb.tile([128, FP], f16, name="dsbt")
            m0t = sb.tile([128, F], u16, name="m0t")
            m90t = sb.tile([128, F], u16, name="m90t")
            s45t = sb.tile([128, F], u16, name="s45t")
            mselt = sb.tile([128, F], f16, name="mselt")
            m45v = sb.tile([128, F], f16, name="m45v")
            m90v = sb.tile([128, F], f16, name="m90v")
            m0v = sb.tile([128, F], f16, name="m0v")

            # --- PSUM: psA x2 halves + psX + psY = 8 banks ---
            psA = [ps.tile([128, HF], f32, name=f"psA{i}")
                   for i in range(2)]
            psX = ps.tile([128, HF], f32, name="psX")
            psY = ps.tile([128, HF], f32, name="psY")

            def pv(t, wpad, lo, hi):
                return t[:].rearrange("p (i w) -> p i w", i=NI)[:, :, lo:hi]

            def fv(ap):
                return ap.rearrange("p (i w) -> p i w", i=NI)

            # --- hoisted one-time setup (outside rep loop) ---
            nc.gpsimd.memset(cbt[:, 0:1], a_ov_b)
            nc.gpsimd.memset(cbt[:, 1:2], b_ov_c)
            nc.gpsimd.memset(pv(Asb, W4, 0, 2), 0.0)
            nc.gpsimd.memset(pv(Asb, W4, W + 2, W + 4), 0.0)
            nc.gpsimd.memset(pv(blurT, W2, 0, 1), 0.0)
            nc.gpsimd.memset(pv(blurT, W2, W + 1, W + 2), 0.0)
            for t in (msq2t, usbt, dsbt):
                nc.gpsimd.memset(pv(t, W2, 0, 1), 0.0)
                nc.gpsimd.memset(pv(t, W2, W + 1, W + 2), 0.0)

            def load(gi):
                ph = gi % 2
                xt = xts[ph]
                if gi < 4:
                    s, r0, r1, _, _ = GROUPS[gi]
                    for c in range(3):
                        nc.sync.dma_start(
                            xt[0:r1 - r0, c * F:(c + 1) * F].rearrange(
                                "p (i w) -> p i w", i=NI),
                            x_d.rearrange("i c h w -> c h i w")[c, r0:r1])
                else:
                    # mini: rows 480..512, partitions = (img, row-in-32)
                    for c in range(3):
                        for i in range(4):
                            nc.sync.dma_start(
                                xt[32 * i:32 * (i + 1),
                                   c * W:(c + 1) * W],
                                x_d[i, c, MINI_R0:H, :])

            def consume_psum(hs, pX, pY):
                """Act/DVE consumption of gx/gy PSUM tiles for one half."""
                nc.scalar.activation(a32[:, hs], pX, Act.Square)
                nc.scalar.activation(c32[:, hs], pY, Act.Square)
                nc.scalar.activation(mA16[:, hs], pX, Act.Square, scale=T1)
                nc.scalar.activation(mB16[:, hs], pX, Act.Square, scale=T2)
                nc.scalar.activation(c16[:, hs], pY, Act.Square)
                nc.scalar.copy(gy16[:, hs], pY)
                nc.vector.tensor_tensor(pxyt[:, hs], pX, gy16[:, hs],
                                        op=Alu.mult)
                nc.vector.tensor_tensor(m0t[:, hs], mA16[:, hs],
                                        c16[:, hs], op=Alu.is_gt)
                nc.vector.tensor_tensor(m90t[:, hs], mB16[:, hs],
                                        c16[:, hs], op=Alu.is_le)
                # msq (Pool)
                nc.gpsimd.tensor_tensor(s4[:, hs], a32[:, hs],
                                        c32[:, hs], op=Alu.add)

            def blur_chain(asb_taps, blur_out, blur_taps, n, w):
                """Pool-only blurH + sobel-horizontal chains.

                asb_taps: [m2, m1, c0, p1, p2] views of padded A
                blur_taps: [m1, c0, p1] views of padded blur
                w(tile): view of a flat scratch tile shaped like the taps
                (fv for full groups, [:, 0:W] for the mini group).
                dxh lands in s1, sht in s3.
                """
                m2, m1, c0, p1, p2 = asb_taps
                nc.gpsimd.tensor_tensor(w(s1), m2, p2, op=Alu.add)
                nc.gpsimd.tensor_tensor(w(s2), m1, p1, op=Alu.add)
                nc.gpsimd.tensor_tensor(s3[:, 0:n], s1[:, 0:n],
                                        bc(0, [128, n]), op=Alu.mult)
                nc.gpsimd.tensor_tensor(s1[:, 0:n], s3[:, 0:n],
                                        s2[:, 0:n], op=Alu.add)
                nc.gpsimd.tensor_tensor(s3[:, 0:n], s1[:, 0:n],
                                        bc(1, [128, n]), op=Alu.mult)
                bm1, bc0, bp1 = blur_taps
                nc.gpsimd.tensor_tensor(blur_out, w(s3), c0, op=Alu.add)
                # sobel horizontal
                nc.gpsimd.tensor_tensor(w(s1), bp1, bm1,
                                        op=Alu.subtract)          # dxh
                nc.gpsimd.tensor_tensor(w(s2), bm1, bp1, op=Alu.add)
                nc.gpsimd.tensor_tensor(w(s4), w(s2), bc0, op=Alu.add)
                nc.gpsimd.tensor_tensor(w(s3), w(s4), bc0, op=Alu.add)

            def front_full(gi):
                ph = gi % 2
                xt = xts[ph]
                # ---- A = (gray o Gv) band mms, halves
                for h in range(2):
                    for i2 in range(2):
                        i = 2 * h + i2
                        for c in range(3):
                            nc.tensor.matmul(
                                psA[h][:, i2 * W:(i2 + 1) * W], wsl(c),
                                xt[:, c * F + i * W:c * F + (i + 1) * W],
                                start=(c == 0), stop=(c == 2))
                    nc.scalar.copy(
                        pv(Asb, W4, 2, W + 2)[:, 2 * h:2 * h + 2, :],
                        psA[h][:].rearrange("p (i w) -> p i w", i=2))

                blur_chain(
                    [pv(Asb, W4, 0, W), pv(Asb, W4, 1, W + 1),
                     pv(Asb, W4, 2, W + 2), pv(Asb, W4, 3, W + 3),
                     pv(Asb, W4, 4, W + 4)],
                    pv(blurT, W2, 1, W + 1),
                    [pv(blurT, W2, 0, W), pv(blurT, W2, 1, W + 1),
                     pv(blurT, W2, 2, W + 2)],
                    F, lambda t: fv(t[:]))

                # ---- gx/gy mms + PSUM consumption per half
                for h in range(2):
                    hs = slice(h * HF, (h + 1) * HF)
                    for i2 in range(2):
                        i = 2 * h + i2
                        nc.tensor.matmul(
                            psX[:, i2 * W:(i2 + 1) * W], wsl(3),
                            s1[:, i * W:(i + 1) * W],
                            start=True, stop=True)
                        nc.tensor.matmul(
                            psY[:, i2 * W:(i2 + 1) * W], wsl(4),
                            s3[:, i * W:(i + 1) * W],
                            start=True, stop=True)
                    consume_psum(hs, psX[:], psY[:])
                # msq2 (Act) + u/d shifts via PE f16 identity bands
                nc.scalar.activation(pv(msq2t, W2, 1, W + 1),
                                     fv(s4[:]), Act.Square)
                for h in range(2):
                    for i2 in range(2):
                        i = 2 * h + i2
                        rhs = msq2t[:, i * W2 + 1:i * W2 + 1 + W]
                        nc.tensor.matmul(
                            psX[:, i2 * W:(i2 + 1) * W],
                            wsl16(0), rhs, start=True, stop=True)
                        nc.tensor.matmul(
                            psY[:, i2 * W:(i2 + 1) * W],
                            wsl16(1), rhs, start=True, stop=True)
                    nc.scalar.copy(
                        pv(usbt, W2, 1, W + 1)[:, 2 * h:2 * h + 2, :],
                        psX[:].rearrange("p (i w) -> p i w", i=2))
                    nc.scalar.copy(
                        pv(dsbt, W2, 1, W + 1)[:, 2 * h:2 * h + 2, :],
                        psY[:].rearrange("p (i w) -> p i w", i=2))
                nc.vector.tensor_scalar(s45t[:], pxyt[:], 0.0, None,
                                        op0=Alu.is_ge)

            def front_mini(gi):
                ph = gi % 2
                xt = xts[ph]
                Am = Asb[:, 0:W4]
                blm = blurT[:, 0:W2]
                psAm = psA[0][:, 0:W]
                psXm = psX[:, 0:W]
                psYm = psY[:, 0:W]
                for c in range(3):
                    nc.tensor.matmul(
                        psAm, wsl(5 + c), xt[:, c * W:(c + 1) * W],
                        start=(c == 0), stop=(c == 2))
                nc.scalar.copy(Am[:, 2:W + 2], psAm)
                blur_chain(
                    [Am[:, 0:W], Am[:, 1:W + 1], Am[:, 2:W + 2],
                     Am[:, 3:W + 3], Am[:, 4:W + 4]],
                    blm[:, 1:W + 1],
                    [blm[:, 0:W], blm[:, 1:W + 1], blm[:, 2:W + 2]],
                    W, lambda t: t[:, 0:W])
                nc.tensor.matmul(psXm, wsl(8), s1[:, 0:W],
                                 start=True, stop=True)
                nc.tensor.matmul(psYm, wsl(9), s3[:, 0:W],
                                 start=True, stop=True)
                consume_psum(slice(0, W), psXm, psYm)
                nc.scalar.activation(msq2t[:, 1:W + 1], s4[:, 0:W],
                                     Act.Square)
                rhs = msq2t[:, 1:W + 1]
                nc.tensor.matmul(psXm, wsl16(2), rhs,
                                 start=True, stop=True)
                nc.tensor.matmul(psYm, wsl16(3), rhs,
                                 start=True, stop=True)
                nc.scalar.copy(usbt[:, 1:W + 1], psXm)
                nc.scalar.copy(dsbt[:, 1:W + 1], psYm)
                nc.vector.tensor_scalar(s45t[:, 0:W], pxyt[:, 0:W],
                                        0.0, None, op0=Alu.is_ge)

            def back(gi):
                if gi < 4:
                    s, r0, r1, zp0, zp1 = GROUPS[gi]
                    nc.vector.tensor_tensor(
                        fv(mselt[:]), pv(usbt, W2, 0, W),
                        pv(dsbt, W2, 2, W + 2), op=Alu.max)  # ul/dr (135)
                    nc.vector.tensor_tensor(
                        fv(m45v[:]), pv(dsbt, W2, 0, W),
                        pv(usbt, W2, 2, W + 2), op=Alu.max)  # dl/ur (45)
                    nc.vector.tensor_tensor(
                        fv(m90v[:]), pv(usbt, W2, 1, W + 1),
                        pv(dsbt, W2, 1, W + 1), op=Alu.max)
                    nc.vector.tensor_tensor(
                        fv(m0v[:]), pv(msq2t, W2, 0, W),
                        pv(msq2t, W2, 2, W + 2), op=Alu.max)
                    nc.vector.copy_predicated(mselt[:], s45t[:], m45v[:])
                    nc.vector.copy_predicated(mselt[:], m90t[:], m90v[:])
                    nc.vector.copy_predicated(mselt[:], m0t[:], m0v[:])
                    nc.vector.tensor_tensor(
                        fv(m45v[:]), pv(msq2t, W2, 1, W + 1),
                        fv(mselt[:]), op=Alu.is_ge)
                    zr0, zr1 = s + zp0, s + zp1
                    for i in range(NI):
                        nc.sync.dma_start(
                            y_d[i, zr0:zr1, :],
                            m45v[zp0:zp1, i * W:(i + 1) * W])
                else:
                    nc.vector.tensor_tensor(
                        mselt[:, 0:W], usbt[:, 0:W], dsbt[:, 2:W + 2],
                        op=Alu.max)
                    nc.vector.tensor_tensor(
                        m45v[:, 0:W], dsbt[:, 0:W], usbt[:, 2:W + 2],
                        op=Alu.max)
                    nc.vector.tensor_tensor(
                        m90v[:, 0:W], usbt[:, 1:W + 1], dsbt[:, 1:W + 1],
                        op=Alu.max)
                    nc.vector.tensor_tensor(
                        m0v[:, 0:W], msq2t[:, 0:W], msq2t[:, 2:W + 2],
                        op=Alu.max)
                    nc.vector.copy_predicated(mselt[:, 0:W], s45t[:, 0:W],
                                              m45v[:, 0:W])
                    nc.vector.copy_predicated(mselt[:, 0:W], m90t[:, 0:W],
                                              m90v[:, 0:W])
                    nc.vector.copy_predicated(mselt[:, 0:W], m0t[:, 0:W],
                                              m0v[:, 0:W])
                    nc.vector.tensor_tensor(
                        m45v[:, 0:W], msq2t[:, 1:W + 1], mselt[:, 0:W],
                        op=Alu.is_ge)
                    for i in range(NI):
                        nc.sync.dma_start(
                            y_d[i, MINI_R0 + 4:H, :],
                            m45v[32 * i + 4:32 * (i + 1), 0:W])

            def front(gi):
                if gi < 4:
                    front_full(gi)
                else:
                    front_mini(gi)

            import contextlib as _ctl
            rep_ctx = (tc.For_i(0, n_reps, 1) if n_reps > 1
                       else _ctl.nullcontext())
            with rep_ctx:
                # pipeline: B(g-1); F(g); load(g+1)
                load(0)
                for g in range(NG + 1):
                    if g >= 1:
                        back(g - 1)
                    if g < NG:
                        front(g)
                    if g + 1 < NG:
                        load(g + 1)
    nc.compile()
    return nc


def _get_nc(n_reps):
    if n_reps not in _NC_CACHE:
        _NC_CACHE[n_reps] = _build(n_reps)
    return _NC_CACHE[n_reps]


def run_on_cores(x, n_reps=1):
    from concourse.bass_utils import run_bass_kernel_spmd

    nc = _get_nc(n_reps)
    w32 = _weights32()
    w16 = _weights16()
    zeros = np.zeros((8, 3 * F), np.float32)
    x = np.ascontiguousarray(np.asarray(x), dtype=np.float32)
    in_maps = [
        {"x": x[c * IMGS_PER_CORE:(c + 1) * IMGS_PER_CORE],
         "w32": w32, "w16": w16, "zeros": zeros}
        for c in range(N_CORES)
    ]
    res = run_bass_kernel_spmd(nc, in_maps, list(range(N_CORES)))
    out = np.concatenate(
        [np.asarray(res.results[c]["y"], dtype=np.float32)[:, None]
         for c in range(N_CORES)], axis=0)
    return out


def kernel(x):
    return run_on_cores(x, n_reps=1)


# revision 14
# speedup vs baseline: 1.5257x; 1.3207x over previous
"""Canny NMS kernel V13e for trn2, 8-core data parallel.

Structure vs V11 (280us baseline):
- u/d row shifts of msq2 via SBUF->SBUF partition-shifted DMA (DMA is
  exempt from the start-partition {0,32,64,96} rule) instead of PE f16
  identity matmuls + Act copies: -4.6us Act/group, -PSUM serialization.
- scalar_tensor_tensor ops (r1, blur, sht chains) decomposed into Pool
  tensor_tensor mult-by-broadcast-const + add: moves ~6.6us/group from
  DVE (the bottleneck) to Pool (which only supports f32 tt arith).
- msq = gx^2+gy^2 from two f32 Act squares + one Pool add; msq2 =
  Square(msq)->f16 (Act). Masks from single-rounded f16 scaled squares
  (Act): mA16=(T1gx)^2, mB16=(T2gx)^2, c16=gy^2; m0/m90 DVE f16 tt.
- Engine split/group: Pool 11 f32 tt (18.8us) | Act 8 (18us) | DVE:
  m0,m90,pxy,s45,4 maxes,3 copy_predicated,z (17.5us) | PE 20 fp32 mm
  (17.1us).  (fp32r matmuls tested: 4x faster but E8M11 rounding of
  sht explodes flips via gy cancellation -- rejected.)
- Last 28 output rows (484..512) as a "mini" group: 4 imgs x 32-row
  blocks on the partition dim, free dim 512+pads (~1/4 group cost);
  block-diagonal band matrices; block-end zero guards via DMA from a
  zeros dram tensor.
- Pipeline: back(g-1); front(g); load(g+1); single-buffered NMS tiles.
"""

import numpy as np

H = W = 512
B = 32
N_CORES = 8
IMGS_PER_CORE = B // N_CORES  # 4

# full groups: (s, load_r0, load_r1, z_p0, z_p1)
GROUPS = [
    (0, 0, 128, 0, 124),      # z rows [0, 124)
    (120, 120, 248, 4, 124),  # [124, 244)
    (240, 240, 368, 4, 124),  # [244, 364)
    (360, 360, 488, 4, 124),  # [364, 484)
]
MINI_R0 = 480                 # mini group: rows 480..512, z [484, 512)
NG = 5

_GRAY_W = (0.299, 0.587, 0.114)
T1 = float(np.float32(np.tan(np.deg2rad(22.5))))
T2 = float(np.float32(np.tan(np.deg2rad(67.5))))

NI = IMGS_PER_CORE  # 4
F = NI * W          # 2048
HF = F // 2         # 1024
W2, W4 = W + 2, W + 4
FP = NI * W2        # 2056
FP2 = NI * W4       # 2064

_NC_CACHE = {}


def _g1n():
    ax = np.arange(-2, 3, dtype=np.float64)
    g = np.exp(-(ax * ax) / 2.0)
    return g / g.sum()


def _band(w, off, n=128):
    Bm = np.zeros((n, n), np.float32)
    idx = np.arange(n)
    for d, wv in enumerate(w):
        kk = idx + d - off
        valid = (kk >= 0) & (kk < n)
        Bm[kk[valid], idx[valid]] = np.float32(wv)
    return Bm


def _band_blocks(w, off, nb=4, bs=32):
    """Block-diagonal band matrix: nb independent bs-row blocks."""
    Bm = np.zeros((nb * bs, nb * bs), np.float32)
    blk = _band(w, off, bs)
    for b in range(nb):
        Bm[b * bs:(b + 1) * bs, b * bs:(b + 1) * bs] = blk
    return Bm


def _weights32():
    g1 = _g1n()
    return np.stack([
        _band(_GRAY_W[0] * g1, 2),          # 0: ch0 gray+Gv
        _band(_GRAY_W[1] * g1, 2),          # 1: ch1
        _band(_GRAY_W[2] * g1, 2),          # 2: ch2
        _band([1.0, 2.0, 1.0], 1),          # 3: Sv (gx vertical)
        _band([-1.0, 0.0, 1.0], 1),         # 4: Dv (gy vertical)
        _band_blocks(_GRAY_W[0] * g1, 2),   # 5: mini ch0
        _band_blocks(_GRAY_W[1] * g1, 2),   # 6: mini ch1
        _band_blocks(_GRAY_W[2] * g1, 2),   # 7: mini ch2
        _band_blocks([1.0, 2.0, 1.0], 1),   # 8: mini Sv
        _band_blocks([-1.0, 0.0, 1.0], 1),  # 9: mini Dv
    ]).astype(np.float32)


def _weights16():
    return np.stack([
        _band([1.0], 1),               # 0: shift up   u[m] = in[m-1]
        _band([1.0], -1),              # 1: shift down d[m] = in[m+1]
        _band_blocks([1.0], 1),        # 2: mini shift up (block-diag)
        _band_blocks([1.0], -1),       # 3: mini shift down (block-diag)
    ]).astype(np.float16)


def _build(n_reps):
    import concourse.bacc as bacc
    import concourse.tile as tile
    from concourse import mybir

    f32 = mybir.dt.float32
    f16 = mybir.dt.float16
    u16 = mybir.dt.uint16
    Alu = mybir.AluOpType
    Act = mybir.ActivationFunctionType

    g1 = _g1n()
    a_ov_b = float(np.float32(g1[0] / g1[1]))
    b_ov_c = float(np.float32(g1[1] / g1[2]))

    nc = bacc.Bacc("TRN2", target_bir_lowering=False, debug=False,
                   num_devices=N_CORES)
    x_d = nc.dram_tensor("x", [NI, 3, H, W], f32,
                         kind="ExternalInput").ap()
    w32_d = nc.dram_tensor("w32", [10, 128, 128], f32,
                           kind="ExternalInput").ap()
    w16_d = nc.dram_tensor("w16", [4, 128, 128], f16,
                           kind="ExternalInput").ap()
    z_d = nc.dram_tensor("zeros", [8, 3 * F], f32, kind="ExternalInput").ap()
    y_d = nc.dram_tensor("y", [NI, H, W], f16, kind="ExternalOutput").ap()

    with tile.TileContext(nc) as tc:
        import contextlib
        with contextlib.ExitStack() as ctx:
            wpool = ctx.enter_context(tc.tile_pool(name="w", bufs=1))
            sb = ctx.enter_context(tc.tile_pool(name="sb", bufs=1))
            ps = ctx.enter_context(tc.tile_pool(name="ps", bufs=1,
                                                space="PSUM"))

            wt32 = wpool.tile([128, 10 * 128], f32)
            nc.sync.dma_start(
                wt32[:].rearrange("k (n m) -> k n m", n=10),
                w32_d.rearrange("n k m -> k n m"))

            def wsl(n):
                return wt32[:, n * 128:(n + 1) * 128]

            wt16 = wpool.tile([128, 4 * 128], f16)
            nc.sync.dma_start(
                wt16[:].rearrange("k (n m) -> k n m", n=4),
                w16_d.rearrange("n k m -> k n m"))

            def wsl16(n):
                return wt16[:, n * 128:(n + 1) * 128]

            # broadcast scalar constants for Pool tt mults
            cbt = wpool.tile([128, 2], f32)

            def bc(k, shape):
                return cbt[:, k:k + 1].to_broadcast(shape)

            # --- SBUF tiles ---
            xts = [sb.tile([128, 3 * F], f32, name=f"xt{i}")
                   for i in range(2)]
            Asb = sb.tile([128, FP2], f32, name="Asb")
            blurT = sb.tile([128, FP], f32, name="blurT")
            s1 = sb.tile([128, F], f32, name="s1")
            s2 = sb.tile([128, F], f32, name="s2")
            s3 = sb.tile([128, F], f32, name="s3")
            s4 = sb.tile([128, F], f32, name="s4")
            gy16 = sb.tile([128, F], f16, name="gy16")
            a32 = sb.tile([128, F], f32, name="a32")
            c32 = sb.tile([128, F], f32, name="c32")
            mA16 = sb.tile([128, F], f16, name="mA16")
            mB16 = sb.tile([128, F], f16, name="mB16")
            c16 = sb.tile([128, F], f16, name="c16")
            pxyt = sb.tile([128, F], f16, name="pxyt")
            msq2t = sb.tile([128, FP], f16, name="msq2t")
            usbt = sb.tile([128, FP], f16, name="usbt")
            dsbt = sb.tile([128, FP], f16, name="dsbt")
            m0t = sb.tile([128, F], u16, name="m0t")
            m90t = sb.tile([128, F], u16, name="m90t")
            s45t = sb.tile([128, F], u16, name="s45t")
            mselt = sb.tile([128, F], f16, name="mselt")
            m45v = sb.tile([128, F], f16, name="m45v")
            m90v = sb.tile([128, F], f16, name="m90v")
            m0v = sb.tile([128, F], f16, name="m0v")

            # --- PSUM: psA x2 halves + psX + psY = 8 banks ---
            psA = [ps.tile([128, HF], f32, name=f"psA{i}")
                   for i in range(2)]
            psX = ps.tile([128, HF], f32, name="psX")
            psY = ps.tile([128, HF], f32, name="psY")

            def pv(t, wpad, lo, hi):
                return t[:].rearrange("p (i w) -> p i w", i=NI)[:, :, lo:hi]

            def fv(ap):
                return ap.rearrange("p (i w) -> p i w", i=NI)

            # --- hoisted one-time setup (outside rep loop) ---
            nc.gpsimd.memset(cbt[:, 0:1], a_ov_b)
            nc.gpsimd.memset(cbt[:, 1:2], b_ov_c)
            nc.gpsimd.memset(pv(Asb, W4, 0, 2), 0.0)
            nc.gpsimd.memset(pv(Asb, W4, W + 2, W + 4), 0.0)
            nc.gpsimd.memset(pv(blurT, W2, 0, 1), 0.0)
            nc.gpsimd.memset(pv(blurT, W2, W + 1, W + 2), 0.0)
            for t in (msq2t, usbt, dsbt):
                nc.gpsimd.memset(pv(t, W2, 0, 1), 0.0)
                nc.gpsimd.memset(pv(t, W2, W + 1, W + 2), 0.0)

            def load(gi):
                ph = gi % 2
                xt = xts[ph]
                if gi < 4:
                    s, r0, r1, _, _ = GROUPS[gi]
                    for c in range(3):
                        nc.sync.dma_start(
                            xt[0:r1 - r0, c * F:(c + 1) * F].rearrange(
                                "p (i w) -> p i w", i=NI),
                            x_d.rearrange("i c h w -> c h i w")[c, r0:r1])
                else:
                    # mini: rows 480..512, partitions = (img, row-in-32)
                    for c in range(3):
                        for i in range(4):
                            nc.sync.dma_start(
                                xt[32 * i:32 * (i + 1),
                                   c * W:(c + 1) * W],
                                x_d[i, c, MINI_R0:H, :])

            def consume_psum(hs, pX, pY):
                """Act/DVE consumption of gx/gy PSUM tiles for one half."""
                nc.scalar.activation(a32[:, hs], pX, Act.Square)
                nc.scalar.activation(c32[:, hs], pY, Act.Square)
                nc.scalar.activation(mA16[:, hs], pX, Act.Square, scale=T1)
                nc.scalar.activation(mB16[:, hs], pX, Act.Square, scale=T2)
                nc.scalar.activation(c16[:, hs], pY, Act.Square)
                nc.scalar.copy(gy16[:, hs], pY)
                nc.vector.tensor_tensor(pxyt[:, hs], pX, gy16[:, hs],
                                        op=Alu.mult)
                nc.vector.tensor_tensor(m0t[:, hs], mA16[:, hs],
                                        c16[:, hs], op=Alu.is_gt)
                nc.vector.tensor_tensor(m90t[:, hs], mB16[:, hs],
                                        c16[:, hs], op=Alu.is_le)
                # msq (Pool)
                nc.gpsimd.tensor_tensor(s4[:, hs], a32[:, hs],
                                        c32[:, hs], op=Alu.add)

            def blur_chain(asb_taps, blur_out, blur_taps, n, w):
                """Pool-only blurH + sobel-horizontal chains.

                asb_taps: [m2, m1, c0, p1, p2] views of padded A
                blur_taps: [m1, c0, p1] views of padded blur
                w(tile): view of a flat scratch tile shaped like the taps
                (fv for full groups, [:, 0:W] for the mini group).
                dxh lands in s1, sht in s3.
                """
                m2, m1, c0, p1, p2 = asb_taps
                nc.gpsimd.tensor_tensor(w(s1), m2, p2, op=Alu.add)
                nc.gpsimd.tensor_tensor(w(s2), m1, p1, op=Alu.add)
                nc.vector.scalar_tensor_tensor(
                    s3[:, 0:n], s1[:, 0:n], a_ov_b, s2[:, 0:n],
                    op0=Alu.mult, op1=Alu.add)
                bm1, bc0, bp1 = blur_taps
                nc.vector.scalar_tensor_tensor(
                    blur_out, w(s3), b_ov_c, c0,
                    op0=Alu.mult, op1=Alu.add)
                # sobel horizontal
                nc.gpsimd.tensor_tensor(w(s1), bp1, bm1,
                                        op=Alu.subtract)          # dxh
                nc.gpsimd.tensor_tensor(w(s2), bm1, bp1, op=Alu.add)
                nc.gpsimd.tensor_tensor(w(s4), w(s2), bc0, op=Alu.add)
                nc.gpsimd.tensor_tensor(w(s3), w(s4), bc0, op=Alu.add)

            def front_full(gi):
                ph = gi % 2
                xt = xts[ph]
                # ---- A = (gray o Gv) band mms, halves
                for h in range(2):
                    for i2 in range(2):
                        i = 2 * h + i2
                        for c in range(3):
                            nc.tensor.matmul(
                                psA[h][:, i2 * W:(i2 + 1) * W], wsl(c),
                                xt[:, c * F + i * W:c * F + (i + 1) * W],
                                start=(c == 0), stop=(c == 2))
                    nc.scalar.copy(
                        pv(Asb, W4, 2, W + 2)[:, 2 * h:2 * h + 2, :],
                        psA[h][:].rearrange("p (i w) -> p i w", i=2))

                blur_chain(
                    [pv(Asb, W4, 0, W), pv(Asb, W4, 1, W + 1),
                     pv(Asb, W4, 2, W + 2), pv(Asb, W4, 3, W + 3),
                     pv(Asb, W4, 4, W + 4)],
                    pv(blurT, W2, 1, W + 1),
                    [pv(blurT, W2, 0, W), pv(blurT, W2, 1, W + 1),
                     pv(blurT, W2, 2, W + 2)],
                    F, lambda t: fv(t[:]))

                # ---- gx/gy mms + PSUM consumption per half
                for h in range(2):
                    hs = slice(h * HF, (h + 1) * HF)
                    for i2 in range(2):
                        i = 2 * h + i2
                        nc.tensor.matmul(
                            psX[:, i2 * W:(i2 + 1) * W], wsl(3),
                            s1[:, i * W:(i + 1) * W],
                            start=True, stop=True)
                        nc.tensor.matmul(
                            psY[:, i2 * W:(i2 + 1) * W], wsl(4),
                            s3[:, i * W:(i + 1) * W],
                            start=True, stop=True)
                    consume_psum(hs, psX[:], psY[:])
                # msq2 (Act) + u/d shifts via PE f16 identity bands
                nc.scalar.activation(pv(msq2t, W2, 1, W + 1),
                                     fv(s4[:]), Act.Square)
                for h in range(2):
                    for i2 in range(2):
                        i = 2 * h + i2
                        rhs = msq2t[:, i * W2 + 1:i * W2 + 1 + W]
                        nc.tensor.matmul(
                            psX[:, i2 * W:(i2 + 1) * W],
                            wsl16(0), rhs, start=True, stop=True)
                        nc.tensor.matmul(
                            psY[:, i2 * W:(i2 + 1) * W],
                            wsl16(1), rhs, start=True, stop=True)
                    nc.scalar.copy(
                        pv(usbt, W2, 1, W + 1)[:, 2 * h:2 * h + 2, :],
                        psX[:].rearrange("p (i w) -> p i w", i=2))
                    nc.scalar.copy(
                        pv(dsbt, W2, 1, W + 1)[:, 2 * h:2 * h + 2, :],
                        psY[:].rearrange("p (i w) -> p i w", i=2))
                nc.vector.tensor_scalar(s45t[:], pxyt[:], 0.0, None,
                                        op0=Alu.is_ge)

            def front_mini(gi):
                ph = gi % 2
                xt = xts[ph]
                Am = Asb[:, 0:W4]
                blm = blurT[:, 0:W2]
                psAm = psA[0][:, 0:W]
                psXm = psX[:, 0:W]
                psYm = psY[:, 0:W]
                for c in range(3):
                    nc.tensor.matmul(
                        psAm, wsl(5 + c), xt[:, c * W:(c + 1) * W],
                        start=(c == 0), stop=(c == 2))
                nc.scalar.copy(Am[:, 2:W + 2], psAm)
                blur_chain(
                    [Am[:, 0:W], Am[:, 1:W + 1], Am[:, 2:W + 2],
                     Am[:, 3:W + 3], Am[:, 4:W + 4]],
                    blm[:, 1:W + 1],
                    [blm[:, 0:W], blm[:, 1:W + 1], blm[:, 2:W + 2]],
                    W, lambda t: t[:, 0:W])
                nc.tensor.matmul(psXm, wsl(8), s1[:, 0:W],
                                 start=True, stop=True)
                nc.tensor.matmul(psYm, wsl(9), s3[:, 0:W],
                                 start=True, stop=True)
                consume_psum(slice(0, W), psXm, psYm)
                nc.scalar.activation(msq2t[:, 1:W + 1], s4[:, 0:W],
                                     Act.Square)
                rhs = msq2t[:, 1:W + 1]
                nc.tensor.matmul(psXm, wsl16(2), rhs,
                                 start=True, stop=True)
                nc.tensor.matmul(psYm, wsl16(3), rhs,
                                 start=True, stop=True)
                nc.scalar.copy(usbt[:, 1:W + 1], psXm)
                nc.scalar.copy(dsbt[:, 1:W + 1], psYm)
                nc.vector.tensor_scalar(s45t[:, 0:W], pxyt[:, 0:W],
                                        0.0, None, op0=Alu.is_ge)

            def back(gi):
                if gi < 4:
                    s, r0, r1, zp0, zp1 = GROUPS[gi]
                    nc.vector.tensor_tensor(
                        fv(mselt[:]), pv(usbt, W2, 0, W),
                        pv(dsbt, W2, 2, W + 2), op=Alu.max)  # ul/dr (135)
                    nc.vector.tensor_tensor(
                        fv(m45v[:]), pv(dsbt, W2, 0, W),
                        pv(usbt, W2, 2, W + 2), op=Alu.max)  # dl/ur (45)
                    nc.vector.tensor_tensor(
                        fv(m90v[:]), pv(usbt, W2, 1, W + 1),
                        pv(dsbt, W2, 1, W + 1), op=Alu.max)
                    nc.vector.tensor_tensor(
                        fv(m0v[:]), pv(msq2t, W2, 0, W),
                        pv(msq2t, W2, 2, W + 2), op=Alu.max)
                    nc.vector.copy_predicated(mselt[:], s45t[:], m45v[:])
                    nc.vector.copy_predicated(mselt[:], m90t[:], m90v[:])
                    nc.vector.copy_predicated(mselt[:], m0t[:], m0v[:])
                    nc.vector.tensor_tensor(
                        fv(m45v[:]), pv(msq2t, W2, 1, W + 1),
                        fv(mselt[:]), op=Alu.is_ge)
                    zr0, zr1 = s + zp0, s + zp1
                    for i in range(NI):
                        nc.sync.dma_start(
                            y_d[i, zr0:zr1, :],
                            m45v[zp0:zp1, i * W:(i + 1) * W])
                else:
                    nc.vector.tensor_tensor(
                        mselt[:, 0:W], usbt[:, 0:W], dsbt[:, 2:W + 2],
                        op=Alu.max)
                    nc.vector.tensor_tensor(
                        m45v[:, 0:W], dsbt[:, 0:W], usbt[:, 2:W + 2],
                        op=Alu.max)
                    nc.vector.tensor_tensor(
                        m90v[:, 0:W], usbt[:, 1:W + 1], dsbt[:, 1:W + 1],
                        op=Alu.max)
                    nc.vector.tensor_tensor(
                        m0v[:, 0:W], msq2t[:, 0:W], msq2t[:, 2:W + 2],
                        op=Alu.max)
                    nc.vector.copy_predicated(mselt[:, 0:W], s45t[:, 0:W],
                                              m45v[:, 0:W])
                    nc.vector.copy_predicated(mselt[:, 0:W], m90t[:, 0:W],
                                              m90v[:, 0:W])
                    nc.vector.copy_predicated(mselt[:, 0:W], m0t[:, 0:W],
                                              m0v[:, 0:W])
                    nc.vector.tensor_tensor(
                        m45v[:, 0:W], msq2t[:, 1:W + 1], mselt[:, 0:W],
                        op=Alu.is_ge)
                    for i in range(NI):
                        nc.sync.dma_start(
                            y_d[i, MINI_R0 + 4:H, :],
                            m45v[32 * i + 4:32 * (i + 1), 0:W])

            def front(gi):
                if gi < 4:
                    front_full(gi)
                else:
                    front_mini(gi)

            import contextlib as _ctl
            rep_ctx = (tc.For_i(0, n_reps, 1) if n_reps > 1
                       else _ctl.nullcontext())
            with rep_ctx:
                # pipeline: B(g-1); F(g); load(g+1)
                load(0)
                for g in range(NG + 1):
                    if g >= 1:
                        back(g - 1)
                    if g < NG:
                        front(g)
                    if g + 1 < NG:
                        load(g + 1)
    nc.compile()
    return nc


def _get_nc(n_reps):
    if n_reps not in _NC_CACHE:
        _NC_CACHE[n_reps] = _build(n_reps)
    return _NC_CACHE[n_reps]


def run_on_cores(x, n_reps=1):
    from concourse.bass_utils import run_bass_kernel_spmd

    nc = _get_nc(n_reps)
    w32 = _weights32()
    w16 = _weights16()
    zeros = np.zeros((8, 3 * F), np.float32)
    x = np.ascontiguousarray(np.asarray(x), dtype=np.float32)
    in_maps = [
        {"x": x[c * IMGS_PER_CORE:(c + 1) * IMGS_PER_CORE],
         "w32": w32, "w16": w16, "zeros": zeros}
        for c in range(N_CORES)
    ]
    res = run_bass_kernel_spmd(nc, in_maps, list(range(N_CORES)))
    out = np.concatenate(
        [np.asarray(res.results[c]["y"], dtype=np.float32)[:, None]
         for c in range(N_CORES)], axis=0)
    return out


def kernel(x):
    return run_on_cores(x, n_reps=1)
